# revision 1
# baseline (speedup 1.0000x reference)
"""Mistral sliding-window GQA attention + LoRA on 8 trn2 cores.

Sharding: DP2 x TP4. Core c -> batch b=c//4, head-slot s=c%4.
Each core: 8 q heads (2 kv groups of 4), full 2048-token sequence.
All matmuls fp32r (TF32-class, ~1e-4 rel err). Attention computed in
transposed layout (S^T tiles [k,q]), softmax without max subtraction
(scores are O(5)), denominators via ones-matmul, band masks generated
on host as 0/1 multiplicative tiles. Out-projection produces partial^T
[4096, 2048]; ReduceScatter(add) over each 4-core batch group splits
the output-channel axis; host transposes/concats.
"""
import math
from contextlib import ExitStack

import numpy as np

import concourse.bass as bass
import concourse.mybir as mybir
import concourse.tile as tile
from concourse import bacc
from concourse.bass_utils import run_bass_kernel_spmd
from concourse.masks import make_identity

F32 = mybir.dt.float32
F32R = mybir.dt.float32r
AF = mybir.ActivationFunctionType

HID = 4096
S = 2048
D = 128
WIN = 1024
NHQ = 8          # q heads per core
G = 2            # kv groups per core
HG = 4           # q heads per kv group
T = 512          # token chunk (matmul free dim)
NT = S // T      # 4
NHC = HID // 128  # 32 hidden chunks
NKT = S // 128    # 16 k tiles
LORA_R = 16
SCALE = 1.0 / math.sqrt(D)
LORA_SCALING = 2.0
EDGE_D0 = [-384, -256, -128, 0, 640, 768, 896, 1024]
EDGE_IDX = {d0: i for i, d0 in enumerate(EDGE_D0)}


def ktiles_for(q0):
    return [k0 for k0 in range(0, S, 128) if -384 <= q0 - k0 <= 1024]


_CACHE = {}
FLAGS = {"bcast": "gpsimd", "masks": True}


def build_nc(null=False, iters=1, upto="full"):
    key = ("null" if null else "full", iters, upto, tuple(sorted(FLAGS.items())))
    if key in _CACHE:
        return _CACHE[key]
    nc = bacc.Bacc("TRN2", target_bir_lowering=False, debug=False,
                   num_devices=8)
    d = {}
    for name, shape in [
        ("hst", [HID, S]), ("wq", [HID, 1024]), ("wk", [HID, 256]),
        ("wv", [HID, 256]), ("wo", [HID, 1024]), ("aq", [HID, LORA_R]),
        ("bq", [LORA_R, 1024]), ("av", [HID, LORA_R]),
        ("bv", [LORA_R, 256]), ("cost", [64, S]), ("sint", [64, S]),
        ("masks", [8, 128, T]),
    ]:
        d[name] = nc.dram_tensor(name, shape, F32, kind="ExternalInput").ap()
    out = nc.dram_tensor("out", [1024, S], F32, kind="ExternalOutput").ap()

    if null:
        _build_null(nc, d, out)
    else:
        _build_body(nc, d, out, iters, upto)
    nc.compile()
    _CACHE[key] = nc
    return nc


def _build_null(nc, d, out):
    with tile.TileContext(nc) as tc:
        with tc.tile_pool(name="sb", bufs=2) as sb:
            t = sb.tile([128, S], F32)
            nc.sync.dma_start(t[:], d["hst"][0:128, :])
            for i in range(8):
                nc.sync.dma_start(out[128 * i:128 * (i + 1), :], t[:])


def _build_body(nc, d, out, iters=1, upto="full"):
    with tile.TileContext(nc) as tc, ExitStack() as octx:
        cp = octx.enter_context(tc.tile_pool(name="const", bufs=1))
        dp = octx.enter_context(tc.tile_pool(name="dram", bufs=1, space="DRAM"))

        ident = cp.tile([128, 128], F32)
        make_identity(nc, ident[:])
        ones = cp.tile([128, 1], F32)
        nc.gpsimd.memset(ones[:], 1.0)
        ones_r = cp.tile([128, 1], F32R)
        nc.vector.tensor_copy(ones_r[:], ones[:])
        ones_row_f = cp.tile([1, 128], F32)
        nc.gpsimd.memset(ones_row_f[:], 1.0)
        ones_row = cp.tile([1, 128], F32R)
        nc.vector.tensor_copy(ones_row[:], ones_row_f[:])

        # LoRA weights: rounded residents (staging comes later via pst pool)
        aq_r = cp.tile([128, NHC, LORA_R], F32R)
        av_r = cp.tile([128, NHC, LORA_R], F32R)
        bq_r = cp.tile([LORA_R, 1024], F32R)
        bv_r = cp.tile([LORA_R, 256], F32R)

        attn_spill = dp.tile([NHQ, 128, S], F32)
        tm_dram = dp.tile([2, NT, LORA_R, T], F32)
        ag = [dp.tile([4 * HG, 128, S], F32, name=f"ag{g}") for g in range(G)]

        for rep in range(iters):
          _one_rep(nc, tc, d, out, rep, ident, ones_r, ones_row, aq_r, av_r,
                   bq_r, bv_r, attn_spill, tm_dram, ag, upto)


def _one_rep(nc, tc, d, out, rep, ident, ones_r, ones_row, aq_r, av_r,
             bq_r, bv_r, attn_spill, tm_dram, ag, upto="full"):
        pctx = ExitStack()
        pa = pctx.enter_context(tc.tile_pool(name=f"pa{rep}", bufs=1))
        pst = pctx.enter_context(tc.tile_pool(name=f"pstream{rep}", bufs=1))

        if rep == 0:
            # stage + round lora weights through stream tags
            aq_st = pst.tile([128, NHC, LORA_R], F32, tag="hst", bufs=2)
            nc.sync.dma_start(aq_st[:],
                              d["aq"].rearrange("(c p) r -> p c r", p=128))
            nc.vector.tensor_copy(aq_r[:], aq_st[:])
            av_st = pst.tile([128, NHC, LORA_R], F32, tag="hst", bufs=2)
            nc.sync.dma_start(av_st[:],
                              d["av"].rearrange("(c p) r -> p c r", p=128))
            nc.vector.tensor_copy(av_r[:], av_st[:])
            for half in range(2):
                bq_st = pst.tile([LORA_R, T], F32, tag="tms2", bufs=2,
                                 name=f"bqst{half}")
                nc.sync.dma_start(bq_st[:], d["bq"][:, T * half:T * (half + 1)])
                nc.vector.tensor_copy(bq_r[:, T * half:T * (half + 1)], bq_st[:])
            bv_st = pst.tile([LORA_R, T], F32, tag="tms2", bufs=2)
            nc.sync.dma_start(bv_st[0:LORA_R, 0:256], d["bv"][:])
            nc.vector.tensor_copy(bv_r[:], bv_st[0:LORA_R, 0:256])

        qtg = pa.tile([128, HG, S], F32R, tag="qtg")
        ktg = pa.tile([128, S], F32R, tag="ktg")
        vng = pa.tile([128, NKT, 128], F32R, tag="vng")

        def rope_into(ps, cs, sn, dst):
            # dst = ps*cos + rotate_half(ps)*sin, written as f32r
            c1 = pst.tile([128, T], F32, tag="rpc")
            nc.vector.tensor_mul(c1[0:64, :], ps[0:64, :], cs[:])
            nc.vector.tensor_mul(c1[64:128, :], ps[64:128, :], cs[:])
            s1 = pst.tile([128, T], F32, tag="rps")
            nc.vector.tensor_mul(s1[0:64, :], ps[64:128, :], sn[:])
            nc.vector.tensor_mul(s1[64:128, :], ps[0:64, :], sn[:])
            nc.vector.tensor_sub(dst[0:64, :], c1[0:64, :], s1[0:64, :])
            nc.vector.tensor_add(dst[64:128, :], c1[64:128, :], s1[64:128, :])

        for g in range(G):
            # ---------------- projection phase for group g ----------------
            with tc.tile_pool(name=f"w{g}_{rep}", bufs=1) as wp, \
                 tc.tile_pool(name=f"pps{g}_{rep}", bufs=1, space="PSUM") as pps:
                wq_r = wp.tile([128, NHC, 512], F32R, tag="wqr")
                wk_r = wp.tile([128, NHC, 128], F32R, tag="wkr")
                wv_r = wp.tile([128, NHC, 128], F32R, tag="wvr")
                for hc in range(NHC):
                    st = pst.tile([128, 512], F32, tag="wst", bufs=2)
                    nc.sync.dma_start(
                        st[:], d["wq"][128 * hc:128 * (hc + 1),
                                       512 * g:512 * (g + 1)])
                    nc.vector.tensor_copy(wq_r[:, hc, :], st[:])
                    stk = pst.tile([128, 256], F32, tag="wkst", bufs=2)
                    nc.sync.dma_start(
                        stk[:, 0:128], d["wk"][128 * hc:128 * (hc + 1),
                                               128 * g:128 * (g + 1)])
                    nc.sync.dma_start(
                        stk[:, 128:256], d["wv"][128 * hc:128 * (hc + 1),
                                                 128 * g:128 * (g + 1)])
                    nc.vector.tensor_copy(wk_r[:, hc, :], stk[:, 0:128])
                    nc.vector.tensor_copy(wv_r[:, hc, :], stk[:, 128:256])

                for t in range(NT):
                    q0 = t * T
                    qps = [pps.tile([128, T], F32, tag=f"q{i}", name=f"qps{i}")
                           for i in range(HG)]
                    kps = pps.tile([128, T], F32, tag="k")
                    vps = pps.tile([128, T], F32, tag="v")
                    if g == 0:
                        lpq = pps.tile([LORA_R, T], F32, tag="lpq")
                        lpv = pps.tile([LORA_R, T], F32, tag="lpv")
                    for hc in range(NHC):
                        hst_st = pst.tile([128, T], F32, tag="hst", bufs=2)
                        nc.sync.dma_start(
                            hst_st[:], d["hst"][128 * hc:128 * (hc + 1),
                                                q0:q0 + T])
                        hst_r = pst.tile([128, T], F32R, tag="hsr", bufs=2)
                        nc.scalar.copy(hst_r[:], hst_st[:])
                        for i in range(HG):
                            nc.tensor.matmul(
                                qps[i][:], wq_r[:, hc, 128 * i:128 * (i + 1)],
                                hst_r[:], start=(hc == 0), stop=False)
                        nc.tensor.matmul(kps[:], wk_r[:, hc, :], hst_r[:],
                                         start=(hc == 0), stop=(hc == NHC - 1))
                        nc.tensor.matmul(vps[:], wv_r[:, hc, :], hst_r[:],
                                         start=(hc == 0), stop=False)
                        if g == 0:
                            nc.tensor.matmul(lpq[:], aq_r[:, hc, :], hst_r[:],
                                             start=(hc == 0),
                                             stop=(hc == NHC - 1))
                            nc.tensor.matmul(lpv[:], av_r[:, hc, :], hst_r[:],
                                             start=(hc == 0),
                                             stop=(hc == NHC - 1))
                    if g == 0:
                        tmq_sb = pst.tile([LORA_R, T], F32R, tag="tms", bufs=2)
                        nc.vector.tensor_copy(tmq_sb[:], lpq[:])
                        nc.sync.dma_start(tm_dram[0, t], tmq_sb[:].bitcast(F32))
                        tmv_sb = pst.tile([LORA_R, T], F32R, tag="tms", bufs=2)
                        nc.vector.tensor_copy(tmv_sb[:], lpv[:])
                        nc.sync.dma_start(tm_dram[1, t], tmv_sb[:].bitcast(F32))
                    else:
                        tmq_st = pst.tile([LORA_R, T], F32, tag="tms2", bufs=2)
                        nc.sync.dma_start(tmq_st[:], tm_dram[0, t])
                        tmq_sb = pst.tile([LORA_R, T], F32R, tag="tms", bufs=2)
                        nc.vector.tensor_copy(tmq_sb[:], tmq_st[:])
                        tmv_st = pst.tile([LORA_R, T], F32, tag="tms2", bufs=2)
                        nc.sync.dma_start(tmv_st[:], tm_dram[1, t])
                        tmv_sb = pst.tile([LORA_R, T], F32R, tag="tms", bufs=2)
                        nc.vector.tensor_copy(tmv_sb[:], tmv_st[:])
                    # LoRA second stage accumulates into the open psum groups
                    for i in range(HG):
                        hg = g * HG + i
                        nc.tensor.matmul(
                            qps[i][:], bq_r[:, 128 * hg:128 * (hg + 1)],
                            tmq_sb[:], start=False, stop=True)
                    nc.tensor.matmul(vps[:], bv_r[:, 128 * g:128 * (g + 1)],
                                     tmv_sb[:], start=False, stop=True)
                    # epilogues: RoPE for q/k, transpose for v
                    cs = pst.tile([64, T], F32, tag="cost", bufs=2)
                    nc.sync.dma_start(cs[:], d["cost"][:, q0:q0 + T])
                    sn = pst.tile([64, T], F32, tag="sint", bufs=2)
                    nc.sync.dma_start(sn[:], d["sint"][:, q0:q0 + T])
                    for i in range(HG):
                        rope_into(qps[i], cs, sn, qtg[:, i, q0:q0 + T])
                    rope_into(kps, cs, sn, ktg[:, q0:q0 + T])
                    vev = pst.tile([128, T], F32, tag="vev", bufs=1)
                    nc.vector.tensor_copy(vev[:], vps[:])
                    for tt in range(4):
                        vtp = pps.tile([128, 128], F32, tag="lpv")
                        nc.tensor.transpose(
                            vtp[:], vev[:, 128 * tt:128 * (tt + 1)], ident[:])
                        nc.vector.tensor_copy(vng[:, 4 * t + tt, :], vtp[:])

            # ---------------- attention phase for group g ----------------
            if upto == "proj":
                continue
            with tc.tile_pool(name=f"am{g}_{rep}", bufs=1) as amp, \
                 tc.tile_pool(name=f"aps{g}_{rep}", bufs=1, space="PSUM") as aps:
                for i in range(HG):
                    hg = g * HG + i
                    for qc in range(NT):
                        q0 = qc * T
                        kts = ktiles_for(q0)
                        avp = aps.tile([128, T], F32, tag="avps", bufs=2)
                        dnp = aps.tile([1, T], F32, tag="dps", bufs=1)
                        last = len(kts) - 1
                        for ki, k0 in enumerate(kts):
                            sps = aps.tile([128, T], F32, tag="sps", bufs=4)
                            nc.tensor.matmul(
                                sps[:], ktg[:, k0:k0 + 128],
                                qtg[:, i, q0:q0 + T], start=True, stop=True)
                            d0 = q0 - k0
                            at = amp.tile([128, T], F32R, tag="at", bufs=3)
                            nc.scalar.activation(at[:], sps[:], AF.Exp)
                            if d0 in EDGE_IDX and FLAGS["masks"]:
                                # zero where (qq - kk + d0) < 0  (causal)
                                if d0 - 127 < 0:
                                    nc.gpsimd.affine_select(
                                        out=at[:], in_=at[:],
                                        pattern=[[1, T]],
                                        compare_op=mybir.AluOpType.is_ge,
                                        fill=0.0, base=d0,
                                        channel_multiplier=-1)
                                # zero where (qq - kk + d0) > 1023 (window)
                                if d0 + T - 1 > 1023:
                                    nc.gpsimd.affine_select(
                                        out=at[:], in_=at[:],
                                        pattern=[[-1, T]],
                                        compare_op=mybir.AluOpType.is_ge,
                                        fill=0.0, base=1023 - d0,
                                        channel_multiplier=1)
                            nc.tensor.matmul(avp[:], vng[:, k0 // 128, :],
                                             at[:], start=(ki == 0),
                                             stop=(ki == last))
                            nc.tensor.matmul(dnp[:], ones_r[:], at[:],
                                             start=(ki == 0), stop=(ki == last))
                        if FLAGS["bcast"] == "gpsimd":
                            rc = amp.tile([1, T], F32, tag="rc", bufs=1)
                            nc.vector.reciprocal(rc[:], dnp[:])
                            bc = amp.tile([128, T], F32, tag="bc", bufs=2)
                            nc.gpsimd.partition_broadcast(bc[:], rc[:])
                        else:
                            rc = amp.tile([1, T], F32R, tag="rc", bufs=1)
                            with nc.allow_low_precision(reason="fp32r round"):
                                nc.vector.reciprocal(rc[:], dnp[:])
                            bcp = aps.tile([128, T], F32, tag="bcp", bufs=1)
                            nc.tensor.matmul(bcp[:], ones_row[:], rc[:],
                                             start=True, stop=True)
                            bc = amp.tile([128, T], F32, tag="bc", bufs=2)
                            nc.scalar.copy(bc[:], bcp[:])
                        ao = amp.tile([128, T], F32R, tag="ao", bufs=2)
                        nc.vector.tensor_mul(ao[:], avp[:], bc[:])
                        nc.sync.dma_start(attn_spill[hg, :, q0:q0 + T],
                                          ao[:].bitcast(F32))
                if upto == "full":
                    nc.gpsimd.collective_compute(
                        "AllGather", mybir.AluOpType.bypass,
                        replica_groups=[[0, 1, 2, 3], [4, 5, 6, 7]],
                        ins=[attn_spill[HG * g:HG * (g + 1)].opt()],
                        outs=[ag[g].opt()])

        pctx.close()

        # ---------------- output projection (local column slice) ----------------
        with tc.tile_pool(name=f"op{rep}", bufs=1) as op, \
             tc.tile_pool(name=f"ost{rep}", bufs=1) as ost, \
             tc.tile_pool(name=f"ops{rep}", bufs=1, space="PSUM") as opsp:
            wo_r = op.tile([128, 32, 8, 128], F32R)
            for dc in range(32):
                st = ost.tile([128, 1024], F32, tag="wost", bufs=2)
                nc.sync.dma_start(
                    st[:], d["wo"][128 * dc:128 * (dc + 1), :])
                dstv = wo_r[:, dc, :, :].rearrange("p a b -> p (a b)")
                nc.vector.tensor_copy(dstv, st[:])
            # head H (global contraction chunk) -> (src half, ag row)
            def src_of(H):
                return (H % 8) // 4, 4 * (H // 8) + (H % 4)
            halves = [[H for H in range(32) if (H % 8) // 4 == h]
                      for h in range(2)]
            for tt in range(NT):
                ts0 = tt * T
                psums = [opsp.tile([128, T], F32, tag=f"o{oc}", name=f"ops{oc}")
                         for oc in range(8)]
                for half in range(2):
                    atr = {}
                    for j, H in enumerate(halves[half]):
                        ast = ost.tile([128, T], F32, tag=f"ast{j % 4}",
                                       bufs=2, name=f"ast{j}")
                        g_src, row = src_of(H)
                        nc.sync.dma_start(ast[:], ag[g_src][row, :, ts0:ts0 + T])
                        ar = ost.tile([128, T], F32R, tag=f"atr{j}",
                                      name=f"atr{j}")
                        nc.scalar.copy(ar[:], ast[:])
                        atr[H] = ar
                    for oc in range(8):
                        for jj, H in enumerate(halves[half]):
                            nc.tensor.matmul(
                                psums[oc][:], wo_r[:, H, oc, :], atr[H][:],
                                start=(half == 0 and jj == 0),
                                stop=(half == 1 and jj == 15))
                for oc in range(8):
                    ev = ost.tile([128, T], F32, tag="oev", bufs=3,
                                  name=f"ev{oc}")
                    nc.scalar.copy(ev[:], psums[oc][:])
                    nc.sync.dma_start(
                        out[128 * oc:128 * (oc + 1), ts0:ts0 + T], ev[:])


def prep_inputs(inputs):
    hs = np.asarray(inputs["hidden_states"], dtype=np.float32)
    pos = np.asarray(inputs["position_ids"]).astype(np.float64)
    Wq = np.asarray(inputs["Wq"], dtype=np.float32)
    Wk = np.asarray(inputs["Wk"], dtype=np.float32)
    Wv = np.asarray(inputs["Wv"], dtype=np.float32)
    Wo = np.asarray(inputs["Wo"], dtype=np.float32)
    aq = np.asarray(inputs["lora_A_q"], dtype=np.float32)
    bq = np.asarray(inputs["lora_B_q"], dtype=np.float32)
    av = np.asarray(inputs["lora_A_v"], dtype=np.float32)
    bv = np.asarray(inputs["lora_B_v"], dtype=np.float32)

    wq_eff = (Wq * SCALE).astype(np.float32)
    bq_eff = (bq * (LORA_SCALING * SCALE)).astype(np.float32)
    bv_eff = (bv * LORA_SCALING).astype(np.float32)

    # RoPE tables per batch, transposed to [d/2, S]
    inv_freq = 1.0 / (10000.0 ** (np.arange(0, D, 2, dtype=np.float64) / D))
    tabs = []
    for b in range(2):
        freqs = np.outer(pos[b], inv_freq)          # [S, 64]
        tabs.append((np.ascontiguousarray(np.cos(freqs).T.astype(np.float32)),
                     np.ascontiguousarray(np.sin(freqs).T.astype(np.float32))))
    hsT = [np.ascontiguousarray(hs[b].T) for b in range(2)]

    # 0/1 edge mask tiles [8, 128, T]
    masks = np.zeros((8, 128, T), dtype=np.float32)
    kk = np.arange(128)[:, None]
    qq = np.arange(T)[None, :]
    for idx, d0 in enumerate(EDGE_D0):
        dd = d0 + qq - kk
        masks[idx] = ((dd >= 0) & (dd < WIN)).astype(np.float32)

    in_maps = []
    for c in range(8):
        b, s = divmod(c, 4)
        cos_b, sin_b = tabs[b]
        in_maps.append({
            "hst": hsT[b],
            "wq": np.ascontiguousarray(wq_eff[:, 1024 * s:1024 * (s + 1)]),
            "wk": np.ascontiguousarray(Wk[:, 256 * s:256 * (s + 1)]),
            "wv": np.ascontiguousarray(Wv[:, 256 * s:256 * (s + 1)]),
            "wo": np.ascontiguousarray(Wo[:, 1024 * s:1024 * (s + 1)]),
            "aq": aq, "av": av,
            "bq": np.ascontiguousarray(bq_eff[:, 1024 * s:1024 * (s + 1)]),
            "bv": np.ascontiguousarray(bv_eff[:, 256 * s:256 * (s + 1)]),
            "cost": cos_b, "sint": sin_b, "masks": masks,
        })
    return in_maps


def assemble(results):
    out = np.empty((2, S, HID), dtype=np.float32)
    for c in range(8):
        b, r = divmod(c, 4)
        out[b, :, 1024 * r:1024 * (r + 1)] = results[c]["out"].T
    return out


def run_prepped(in_maps, null=False, iters=1):
    nc = build_nc(null=null, iters=iters)
    return run_bass_kernel_spmd(nc, in_maps, list(range(8)), trace=False)


def kernel(**inputs) -> np.ndarray:
    in_maps = prep_inputs(inputs)
    res = run_prepped(in_maps)
    return assemble(res.results)



# revision 2
# speedup vs baseline: 239.7007x; 239.7007x over previous
"""Mistral sliding-window GQA attention + LoRA on 8 trn2 cores — v2.

Sharding: DP2 x TP4. Core c -> batch b=c//4, head-slot s=c%4.
Each core: 8 q heads (2 kv groups of 4), full 2048-token sequence.

v2 layout: bf16 weights + activations (psum accumulates fp32), single pass
over hidden_states (resident 4MB bf16 chunk, two psum passes), attention
interleaved per 512-token chunk against an appended K/V cache, per-chunk
AllGather of bf16 attention outputs overlapped with later chunks, and the
out-projection of chunks 0-2 overlapping the final gather. Scores/q/k stay
fp32r for precision; exp/softmax denominator via ones-matmul as before.
"""
import math
from contextlib import ExitStack

import numpy as np
import ml_dtypes

import concourse.bass as bass
import concourse.mybir as mybir
import concourse.tile as tile
from concourse import bacc
from concourse.bass_utils import run_bass_kernel_spmd
from concourse.masks import make_identity

F32 = mybir.dt.float32
F32R = mybir.dt.float32r
BF16 = mybir.dt.bfloat16
AF = mybir.ActivationFunctionType
BFDT = ml_dtypes.bfloat16

HID = 4096
S = 2048
D = 128
WIN = 1024
NHQ = 8          # q heads per core
G = 2            # kv groups per core
HG = 4           # q heads per kv group
T = 512          # token chunk
NT = S // T      # 4
NHC = HID // 128  # 32 hidden chunks
NKT = S // 128    # 16 k tiles
LORA_R = 16
SCALE = 1.0 / math.sqrt(D)
LORA_SCALING = 2.0


def ktiles_for(q0):
    return [k0 for k0 in range(0, S, 128) if -(T - 128) <= q0 - k0 <= WIN]


_CACHE = {}


def build_nc(null=False, iters=1, upto="full"):
    key = ("null" if null else "full", iters, upto)
    if key in _CACHE:
        return _CACHE[key]
    nc = bacc.Bacc("TRN2", target_bir_lowering=False, debug=False,
                   num_devices=8)
    d = {}
    for name, shape, dt in [
        ("hst", [HID, S], BF16), ("wq", [HID, 1024], BF16),
        ("wk", [HID, 256], BF16), ("wv", [HID, 256], BF16),
        ("wo", [HID, 1024], BF16), ("aq", [HID, LORA_R], BF16),
        ("bq", [LORA_R, 1024], BF16), ("av", [HID, LORA_R], BF16),
        ("bv", [LORA_R, 256], BF16), ("cost", [128, S], F32),
        ("sint", [128, S], F32),
    ]:
        d[name] = nc.dram_tensor(name, shape, dt, kind="ExternalInput").ap()
    out = nc.dram_tensor("out", [1024, S], F32, kind="ExternalOutput").ap()

    if null:
        _build_null(nc, d, out)
    else:
        _build_body(nc, d, out, iters, upto)
    nc.compile()
    _CACHE[key] = nc
    return nc


def _build_null(nc, d, out):
    with tile.TileContext(nc) as tc:
        with tc.tile_pool(name="sb", bufs=2) as sb:
            t = sb.tile([64, S], F32)
            nc.sync.dma_start(t[:], d["cost"][0:64, :])
            for i in range(8):
                nc.sync.dma_start(out[128 * i:128 * i + 64, :], t[:])


def _build_body(nc, d, out, iters=1, upto="full"):
    with tile.TileContext(nc) as tc, ExitStack() as octx:
        cp = octx.enter_context(tc.tile_pool(name="const", bufs=1))
        dp = octx.enter_context(tc.tile_pool(name="dram", bufs=1, space="DRAM"))

        ident = cp.tile([128, 128], BF16)
        make_identity(nc, ident[:])
        ones_bf = cp.tile([128, 1], BF16)
        nc.gpsimd.memset(ones_bf[:], 1.0)

        spill = dp.tile([NT, NHQ, 128, T], BF16)
        ag = dp.tile([NT, 4 * NHQ, 128, T], BF16)

        for rep in range(iters):
            _one_rep(nc, tc, d, out, rep, ident, ones_bf, spill, ag, upto)


def _one_rep(nc, tc, d, out, rep, ident, ones_bf, spill, ag, upto="full"):
    # LIFO pool stack: sp (whole rep) < pa (until out-proj) < wp (until chunk 3)
    spctx = ExitStack()
    sp = spctx.enter_context(tc.tile_pool(name=f"sp{rep}", bufs=1))
    pactx = ExitStack()
    pa = pactx.enter_context(tc.tile_pool(name=f"pa{rep}", bufs=1))
    wctx = ExitStack()
    wp = wctx.enter_context(tc.tile_pool(name=f"w{rep}", bufs=1))
    opctx = ExitStack()

    # hst chunk 0 first so the first matmuls can start early; then weights.
    hck = wp.tile([128, NHC, T], BF16, tag="hck")
    hst_p = d["hst"].rearrange("(c p) s -> p c s", p=128)
    nc.sync.dma_start(hck[:, :, :], hst_p[:, :, 0:T])

    wq_r = wp.tile([128, NHC, 1024], BF16)
    wq_p = d["wq"].rearrange("(c p) n -> p c n", p=128)
    wk_r = wp.tile([128, NHC, 256], BF16)
    wk_p = d["wk"].rearrange("(c p) n -> p c n", p=128)
    wv_r = wp.tile([128, NHC, 256], BF16)
    wv_p = d["wv"].rearrange("(c p) n -> p c n", p=128)
    for cc in range(0, NHC, 8):
        nc.sync.dma_start(wq_r[:, cc:cc + 8, :], wq_p[:, cc:cc + 8, :])
        nc.sync.dma_start(wk_r[:, cc:cc + 8, :], wk_p[:, cc:cc + 8, :])
        nc.sync.dma_start(wv_r[:, cc:cc + 8, :], wv_p[:, cc:cc + 8, :])
    aq_r = wp.tile([128, NHC, LORA_R], BF16)
    nc.sync.dma_start(aq_r[:], d["aq"].rearrange("(c p) r -> p c r", p=128))
    av_r = wp.tile([128, NHC, LORA_R], BF16)
    nc.sync.dma_start(av_r[:], d["av"].rearrange("(c p) r -> p c r", p=128))
    bq_r = wp.tile([LORA_R, 1024], BF16)
    nc.sync.dma_start(bq_r[:], d["bq"][:, :])
    # rows 32:48 so the lora-v second-stage matmul shares tm's base partition
    bv_r = wp.tile([48, 256], BF16)
    nc.sync.dma_start(bv_r[32:48, :], d["bv"][:, :])

    # persistent per-rep activation state
    qcur = pa.tile([128, NHQ, T], F32R, name="qcur")
    ktg = pa.tile([128, G, S], F32R, name="ktg")
    vng = pa.tile([128, G, NKT, 128], BF16, name="vng")

    def rope_into(ps, cs, sn, dst):
        # dst = ps*cos + rotate_half(ps)*sin, psum f32 in, f32r out.
        # Stage psum -> SBUF on ACT first so the bank frees fast; the DVE
        # rope chain then runs off the critical path.
        st = sp.tile([128, T], F32, tag="rst", bufs=2)
        nc.scalar.copy(st[:], ps[:])
        c1 = sp.tile([128, T], F32, tag="rpc", bufs=1)
        nc.vector.tensor_mul(c1[0:64, :], st[0:64, :], cs[0:64, :])
        nc.vector.tensor_mul(c1[64:128, :], st[64:128, :], cs[64:128, :])
        s1 = sp.tile([128, T], F32, tag="rps", bufs=1)
        nc.vector.tensor_mul(s1[0:64, :], st[64:128, :], sn[64:128, :])
        nc.vector.tensor_mul(s1[64:128, :], st[0:64, :], sn[0:64, :])
        nc.vector.tensor_sub(dst[0:64, :], c1[0:64, :], s1[0:64, :])
        nc.vector.tensor_add(dst[64:128, :], c1[64:128, :], s1[64:128, :])

    for t in range(NT):
        q0 = t * T
        cs = sp.tile([128, T], F32, tag="cs", bufs=1)
        nc.sync.dma_start(cs[:], d["cost"][:, q0:q0 + T])
        sn = sp.tile([128, T], F32, tag="sn", bufs=1)
        nc.sync.dma_start(sn[:], d["sint"][:, q0:q0 + T])

        with tc.tile_pool(name=f"pp{rep}_{t}", bufs=1, space="PSUM") as pp:
            tm = sp.tile([48, T], BF16, tag="tm", bufs=2)
            for pas in range(2):  # pass A: heads 0-3 + kv g0 (+lora), B: 4-7 + g1
                g = pas
                qps = [pp.tile([128, T], F32, tag=f"q{i}", name=f"qps{i}")
                       for i in range(HG)]
                kps = pp.tile([128, T], F32, tag="k")
                vps = pp.tile([128, T], F32, tag="v")
                if pas == 0:
                    lps = pp.tile([48, T], F32, tag="l")
                for hc in range(NHC):
                    h = hck[:, hc, :]
                    for i in range(HG):
                        nc.tensor.matmul(
                            qps[i][:], wq_r[:, hc, 512 * g + 128 * i:
                                            512 * g + 128 * (i + 1)],
                            h, start=(hc == 0), stop=False)
                    nc.tensor.matmul(kps[:], wk_r[:, hc, 128 * g:128 * (g + 1)],
                                     h, start=(hc == 0), stop=(hc == NHC - 1))
                    nc.tensor.matmul(vps[:], wv_r[:, hc, 128 * g:128 * (g + 1)],
                                     h, start=(hc == 0), stop=False)
                    if pas == 0:
                        nc.tensor.matmul(lps[0:16, :], aq_r[:, hc, :], h,
                                         start=(hc == 0), stop=(hc == NHC - 1))
                        nc.tensor.matmul(lps[32:48, :], av_r[:, hc, :], h,
                                         start=(hc == 0), stop=(hc == NHC - 1))
                if pas == 0:
                    nc.vector.tensor_copy(tm[:], lps[:])
                for i in range(HG):
                    hh = g * HG + i
                    nc.tensor.matmul(qps[i][:],
                                     bq_r[:, 128 * hh:128 * (hh + 1)],
                                     tm[0:16, :], start=False, stop=True)
                nc.tensor.matmul(vps[:], bv_r[32:48, 128 * g:128 * (g + 1)],
                                 tm[32:48, :], start=False, stop=True)
                # epilogues: RoPE q/k, transpose v into vng
                for i in range(HG):
                    rope_into(qps[i], cs, sn, qcur[:, g * HG + i, :])
                rope_into(kps, cs, sn, ktg[:, g, q0:q0 + T])
                vev = sp.tile([128, T], BF16, tag="vev", bufs=2)
                nc.vector.tensor_copy(vev[:], vps[:])
                for tt in range(4):
                    vtp = pp.tile([128, 128], BF16, tag="vt")
                    nc.tensor.transpose(
                        vtp[:], vev[:, 128 * tt:128 * (tt + 1)], ident[:])
                    nc.vector.tensor_copy(vng[:, g, 4 * t + tt, :], vtp[:])

        if t < NT - 1:
            # prefetch next hst chunk now: queued ahead of the attention
            # spill DMA, so it starts as soon as pass B's reads finish and
            # lands during chunk t's attention.
            nq0 = q0 + T
            nc.sync.dma_start(hck[:, :, :], hst_p[:, :, nq0:nq0 + T])
        if t == NT - 1:
            wctx.close()   # free wq/wk/wv/hst region for wo + ag readback
            # open the out-proj pool and start its big loads now so they
            # run during chunk 3's attention
            op = opctx.enter_context(tc.tile_pool(name=f"op{rep}", bufs=1))
            wo_r = op.tile([128, NHC, 8, 128], BF16)
            wo_p = d["wo"].rearrange("(c p) n -> p c n", p=128)
            for cc in range(0, NHC, 8):
                dst = wo_r[:, cc:cc + 8, :, :].rearrange("p a b c -> p a (b c)")
                nc.sync.dma_start(dst, wo_p[:, cc:cc + 8, :])
            if upto == "full":
                agb0 = op.tile([128, 4 * NHQ, T], BF16, tag="agb", bufs=2)
                nc.sync.dma_start(agb0[:], ag[0].rearrange("h p s -> p h s"))

        if upto == "proj":
            continue

        # ---------------- attention for chunk t ----------------
        with tc.tile_pool(name=f"ap{rep}_{t}", bufs=1, space="PSUM") as ap:
            spl = sp.tile([128, NHQ, T], BF16, tag="spl", bufs=1)
            for h in range(NHQ):
                g = h // HG
                kts = ktiles_for(q0)
                last = len(kts) - 1
                avp = ap.tile([128, T], F32, tag="avp", bufs=2)
                dnp = ap.tile([1, T], F32, tag="dnp", bufs=2)
                for ki, k0 in enumerate(kts):
                    sps = ap.tile([128, T], F32, tag="sps", bufs=2)
                    nc.tensor.matmul(sps[:], ktg[:, g, k0:k0 + 128],
                                     qcur[:, h, :], start=True, stop=True)
                    at = sp.tile([128, T], BF16, tag="at", bufs=3)
                    nc.scalar.activation(at[:], sps[:], AF.Exp)
                    d0 = q0 - k0
                    if d0 - 127 < 0:
                        # zero where (qq - kk + d0) < 0  (causal)
                        nc.gpsimd.affine_select(
                            out=at[:], in_=at[:], pattern=[[1, T]],
                            compare_op=mybir.AluOpType.is_ge,
                            fill=0.0, base=d0, channel_multiplier=-1)
                    if d0 + T - 1 > WIN - 1:
                        # zero where (qq - kk + d0) > WIN-1 (window)
                        nc.gpsimd.affine_select(
                            out=at[:], in_=at[:], pattern=[[-1, T]],
                            compare_op=mybir.AluOpType.is_ge,
                            fill=0.0, base=WIN - 1 - d0, channel_multiplier=1)
                    nc.tensor.matmul(avp[:], vng[:, g, k0 // 128, :], at[:],
                                     start=(ki == 0), stop=(ki == last))
                    nc.tensor.matmul(dnp[:], ones_bf[:], at[:],
                                     start=(ki == 0), stop=(ki == last))
                rc = sp.tile([1, T], F32, tag="rc", bufs=1)
                nc.vector.reciprocal(rc[:], dnp[:])
                bc = sp.tile([128, T], F32, tag="bc", bufs=1)
                nc.gpsimd.partition_broadcast(bc[:], rc[:])
                nc.vector.tensor_mul(spl[:, h, :], avp[:], bc[:])
            nc.scalar.dma_start(spill[t].rearrange("h p s -> p h s"), spl[:])
            if upto == "full":
                nc.gpsimd.collective_compute(
                    "AllGather", mybir.AluOpType.bypass,
                    replica_groups=[[0, 1, 2, 3], [4, 5, 6, 7]],
                    ins=[spill[t].opt()], outs=[ag[t].opt()])

    if upto != "full":
        opctx.close()
        pactx.close()
        spctx.close()
        return

    # ---------------- output projection ----------------
    with tc.tile_pool(name=f"ops{rep}", bufs=1, space="PSUM") as opp:
        for t in range(NT):
            q0 = t * T
            if t == 0:
                agb = agb0
            else:
                agb = op.tile([128, 4 * NHQ, T], BF16, tag="agb", bufs=2)
                nc.sync.dma_start(agb[:], ag[t].rearrange("h p s -> p h s"))
            psums = [opp.tile([128, T], F32, tag=f"o{oc}", name=f"ops{oc}")
                     for oc in range(8)]
            for H in range(4 * NHQ):
                for oc in range(8):
                    nc.tensor.matmul(psums[oc][:], wo_r[:, H, oc, :],
                                     agb[:, H, :], start=(H == 0),
                                     stop=(H == 4 * NHQ - 1))
            for oc in range(8):
                ev = op.tile([128, T], F32, tag="ev", bufs=2, name=f"ev{oc}")
                nc.scalar.copy(ev[:], psums[oc][:])
                nc.sync.dma_start(out[128 * oc:128 * (oc + 1), q0:q0 + T],
                                  ev[:])
    opctx.close()
    pactx.close()
    spctx.close()


def prep_inputs(inputs):
    hs = np.asarray(inputs["hidden_states"], dtype=np.float32)
    pos = np.asarray(inputs["position_ids"]).astype(np.float64)
    Wq = np.asarray(inputs["Wq"], dtype=np.float32)
    Wk = np.asarray(inputs["Wk"], dtype=np.float32)
    Wv = np.asarray(inputs["Wv"], dtype=np.float32)
    Wo = np.asarray(inputs["Wo"], dtype=np.float32)
    aq = np.asarray(inputs["lora_A_q"], dtype=np.float32)
    bq = np.asarray(inputs["lora_B_q"], dtype=np.float32)
    av = np.asarray(inputs["lora_A_v"], dtype=np.float32)
    bv = np.asarray(inputs["lora_B_v"], dtype=np.float32)

    wq_eff = (Wq * SCALE).astype(BFDT)
    bq_eff = (bq * (LORA_SCALING * SCALE)).astype(BFDT)
    bv_eff = (bv * LORA_SCALING).astype(BFDT)
    wk_b = Wk.astype(BFDT)
    wv_b = Wv.astype(BFDT)
    wo_b = Wo.astype(BFDT)
    aq_b = aq.astype(BFDT)
    av_b = av.astype(BFDT)

    inv_freq = 1.0 / (10000.0 ** (np.arange(0, D, 2, dtype=np.float64) / D))
    tabs = []
    for b in range(2):
        freqs = np.outer(pos[b], inv_freq)          # [S, 64]
        ct = np.cos(freqs).T.astype(np.float32)
        st = np.sin(freqs).T.astype(np.float32)
        # replicated to 128 rows so each rope half reads an aligned copy
        tabs.append((np.ascontiguousarray(np.concatenate([ct, ct], axis=0)),
                     np.ascontiguousarray(np.concatenate([st, st], axis=0))))
    hsT = [np.ascontiguousarray(hs[b].T).astype(BFDT) for b in range(2)]

    in_maps = []
    for c in range(8):
        b, s = divmod(c, 4)
        cos_b, sin_b = tabs[b]
        in_maps.append({
            "hst": hsT[b],
            "wq": np.ascontiguousarray(wq_eff[:, 1024 * s:1024 * (s + 1)]),
            "wk": np.ascontiguousarray(wk_b[:, 256 * s:256 * (s + 1)]),
            "wv": np.ascontiguousarray(wv_b[:, 256 * s:256 * (s + 1)]),
            "wo": np.ascontiguousarray(wo_b[:, 1024 * s:1024 * (s + 1)]),
            "aq": aq_b, "av": av_b,
            "bq": np.ascontiguousarray(bq_eff[:, 1024 * s:1024 * (s + 1)]),
            "bv": np.ascontiguousarray(bv_eff[:, 256 * s:256 * (s + 1)]),
            "cost": cos_b, "sint": sin_b,
        })
    return in_maps


def assemble(results):
    out = np.empty((2, S, HID), dtype=np.float32)
    for c in range(8):
        b, r = divmod(c, 4)
        out[b, :, 1024 * r:1024 * (r + 1)] = results[c]["out"].T
    return out


def run_prepped(in_maps, null=False, iters=1):
    nc = build_nc(null=null, iters=iters)
    return run_bass_kernel_spmd(nc, in_maps, list(range(8)), trace=False)


def kernel(**inputs) -> np.ndarray:
    in_maps = prep_inputs(inputs)
    res = run_prepped(in_maps)
    return assemble(res.results)


# revision 3
# speedup vs baseline: 248.8095x; 1.0380x over previous
"""Mistral sliding-window GQA attention + LoRA on 8 trn2 cores — v2.

Sharding: DP2 x TP4. Core c -> batch b=c//4, head-slot s=c%4.
Each core: 8 q heads (2 kv groups of 4), full 2048-token sequence.

v2 layout: bf16 weights + activations (psum accumulates fp32), single pass
over hidden_states (resident 4MB bf16 chunk, two psum passes), attention
interleaved per 512-token chunk against an appended K/V cache, per-chunk
AllGather of bf16 attention outputs overlapped with later chunks, and the
out-projection of chunks 0-2 overlapping the final gather. Scores/q/k stay
fp32r for precision; exp/softmax denominator via ones-matmul as before.
"""
import math
from contextlib import ExitStack

import numpy as np
import ml_dtypes

import concourse.bass as bass
import concourse.mybir as mybir
import concourse.tile as tile
from concourse import bacc
from concourse.bass_utils import run_bass_kernel_spmd
from concourse.masks import make_identity

F32 = mybir.dt.float32
F32R = mybir.dt.float32r
BF16 = mybir.dt.bfloat16
AF = mybir.ActivationFunctionType
BFDT = ml_dtypes.bfloat16

HID = 4096
S = 2048
D = 128
WIN = 1024
NHQ = 8          # q heads per core
G = 2            # kv groups per core
HG = 4           # q heads per kv group
T = 512          # token chunk
NT = S // T      # 4
NHC = HID // 128  # 32 hidden chunks
NKT = S // 128    # 16 k tiles
LORA_R = 16
SCALE = 1.0 / math.sqrt(D)
LORA_SCALING = 2.0


def ktiles_for(q0):
    return [k0 for k0 in range(0, S, 128) if -(T - 128) <= q0 - k0 <= WIN]


_CACHE = {}


def build_nc(null=False, iters=1, upto="full"):
    key = ("null" if null else "full", iters, upto)
    if key in _CACHE:
        return _CACHE[key]
    nc = bacc.Bacc("TRN2", target_bir_lowering=False, debug=False,
                   num_devices=8)
    d = {}
    for name, shape, dt in [
        ("hst", [HID, S], BF16), ("wq", [HID, 1024], BF16),
        ("wk", [HID, 256], BF16), ("wv", [HID, 256], BF16),
        ("wo", [HID, 1024], BF16), ("aq", [HID, LORA_R], BF16),
        ("bq", [LORA_R, 1024], BF16), ("av", [HID, LORA_R], BF16),
        ("bv", [LORA_R, 256], BF16), ("cost", [128, S], F32),
        ("sint", [128, S], F32),
    ]:
        d[name] = nc.dram_tensor(name, shape, dt, kind="ExternalInput").ap()
    out = nc.dram_tensor("out", [1024, S], F32, kind="ExternalOutput").ap()

    if null:
        _build_null(nc, d, out)
    else:
        _build_body(nc, d, out, iters, upto)
    nc.compile()
    _CACHE[key] = nc
    return nc


def _build_null(nc, d, out):
    with tile.TileContext(nc) as tc:
        with tc.tile_pool(name="sb", bufs=2) as sb:
            t = sb.tile([64, S], F32)
            nc.sync.dma_start(t[:], d["cost"][0:64, :])
            for i in range(8):
                nc.sync.dma_start(out[128 * i:128 * i + 64, :], t[:])


def _build_body(nc, d, out, iters=1, upto="full"):
    with tile.TileContext(nc) as tc, ExitStack() as octx:
        cp = octx.enter_context(tc.tile_pool(name="const", bufs=1))
        dp = octx.enter_context(tc.tile_pool(name="dram", bufs=1, space="DRAM"))

        ident = cp.tile([128, 128], BF16)
        make_identity(nc, ident[:])
        ones_bf = cp.tile([128, 1], BF16)
        nc.gpsimd.memset(ones_bf[:], 1.0)

        spill = dp.tile([NT, NHQ, 128, T], BF16)
        ag = dp.tile([NT, 4 * NHQ, 128, T], BF16)

        for rep in range(iters):
            _one_rep(nc, tc, d, out, rep, ident, ones_bf, spill, ag, upto)


def _one_rep(nc, tc, d, out, rep, ident, ones_bf, spill, ag, upto="full"):
    # LIFO pool stack: sp (whole rep) < pa (until out-proj) < wp (until chunk 3)
    spctx = ExitStack()
    sp = spctx.enter_context(tc.tile_pool(name=f"sp{rep}", bufs=1))
    pactx = ExitStack()
    pa = pactx.enter_context(tc.tile_pool(name=f"pa{rep}", bufs=1))
    wctx = ExitStack()
    wp = wctx.enter_context(tc.tile_pool(name=f"w{rep}", bufs=1))
    opctx = ExitStack()

    # hst chunk 0 first so the first matmuls can start early; then weights.
    hck = wp.tile([128, NHC, T], BF16, tag="hck")
    hst_p = d["hst"].rearrange("(c p) s -> p c s", p=128)
    nc.sync.dma_start(hck[:, :, :], hst_p[:, :, 0:T])

    wq_r = wp.tile([128, NHC, 1024], BF16)
    wq_p = d["wq"].rearrange("(c p) n -> p c n", p=128)
    wk_r = wp.tile([128, NHC, 256], BF16)
    wk_p = d["wk"].rearrange("(c p) n -> p c n", p=128)
    wv_r = wp.tile([128, NHC, 256], BF16)
    wv_p = d["wv"].rearrange("(c p) n -> p c n", p=128)
    for cc in range(0, NHC, 8):
        nc.sync.dma_start(wq_r[:, cc:cc + 8, :], wq_p[:, cc:cc + 8, :])
        nc.sync.dma_start(wk_r[:, cc:cc + 8, :], wk_p[:, cc:cc + 8, :])
        nc.sync.dma_start(wv_r[:, cc:cc + 8, :], wv_p[:, cc:cc + 8, :])
    aq_r = wp.tile([128, NHC, LORA_R], BF16)
    nc.sync.dma_start(aq_r[:], d["aq"].rearrange("(c p) r -> p c r", p=128))
    av_r = wp.tile([128, NHC, LORA_R], BF16)
    nc.sync.dma_start(av_r[:], d["av"].rearrange("(c p) r -> p c r", p=128))
    bq_r = wp.tile([LORA_R, 1024], BF16)
    nc.sync.dma_start(bq_r[:], d["bq"][:, :])
    # rows 32:48 so the lora-v second-stage matmul shares tm's base partition
    bv_r = wp.tile([48, 256], BF16)
    nc.sync.dma_start(bv_r[32:48, :], d["bv"][:, :])

    # persistent per-rep activation state
    qcur = pa.tile([128, NHQ, T], F32R, name="qcur")
    ktg = pa.tile([128, G, S], F32R, name="ktg")
    vng = pa.tile([128, G, NKT, 128], BF16, name="vng")

    def rope_into(ps, cs, sn, dst):
        # dst = ps*cos + rotate_half(ps)*sin, psum f32 in, f32r out.
        # Stage psum -> SBUF on ACT first so the bank frees fast; the DVE
        # rope chain then runs off the critical path.
        st = sp.tile([128, T], F32, tag="rst", bufs=2)
        nc.scalar.copy(st[:], ps[:])
        c1 = sp.tile([128, T], F32, tag="rpc", bufs=1)
        nc.vector.tensor_mul(c1[0:64, :], st[0:64, :], cs[0:64, :])
        nc.vector.tensor_mul(c1[64:128, :], st[64:128, :], cs[64:128, :])
        s1 = sp.tile([128, T], F32, tag="rps", bufs=1)
        nc.vector.tensor_mul(s1[0:64, :], st[64:128, :], sn[64:128, :])
        nc.vector.tensor_mul(s1[64:128, :], st[0:64, :], sn[0:64, :])
        nc.vector.tensor_sub(dst[0:64, :], c1[0:64, :], s1[0:64, :])
        nc.vector.tensor_add(dst[64:128, :], c1[64:128, :], s1[64:128, :])

    for t in range(NT):
        q0 = t * T
        cs = sp.tile([128, T], F32, tag="cs", bufs=1)
        nc.sync.dma_start(cs[:], d["cost"][:, q0:q0 + T])
        sn = sp.tile([128, T], F32, tag="sn", bufs=1)
        nc.sync.dma_start(sn[:], d["sint"][:, q0:q0 + T])

        with tc.tile_pool(name=f"pp{rep}_{t}", bufs=1, space="PSUM") as pp:
            tm = sp.tile([48, T], BF16, tag="tm", bufs=1)
            for pas in range(2):  # pass A: heads 0-3 + kv g0 (+lora), B: 4-7 + g1
                g = pas
                qps = [pp.tile([128, T], F32, tag=f"q{i}", name=f"qps{i}")
                       for i in range(HG)]
                kps = pp.tile([128, T], F32, tag="k")
                vps = pp.tile([128, T], F32, tag="v")
                if pas == 0:
                    lps = pp.tile([48, T], F32, tag="l")
                for hc in range(NHC):
                    h = hck[:, hc, :]
                    for i in range(HG):
                        nc.tensor.matmul(
                            qps[i][:], wq_r[:, hc, 512 * g + 128 * i:
                                            512 * g + 128 * (i + 1)],
                            h, start=(hc == 0), stop=False)
                    nc.tensor.matmul(kps[:], wk_r[:, hc, 128 * g:128 * (g + 1)],
                                     h, start=(hc == 0), stop=(hc == NHC - 1))
                    nc.tensor.matmul(vps[:], wv_r[:, hc, 128 * g:128 * (g + 1)],
                                     h, start=(hc == 0), stop=False)
                    if pas == 0:
                        nc.tensor.matmul(lps[0:16, :], aq_r[:, hc, :], h,
                                         start=(hc == 0), stop=(hc == NHC - 1))
                        nc.tensor.matmul(lps[32:48, :], av_r[:, hc, :], h,
                                         start=(hc == 0), stop=(hc == NHC - 1))
                if pas == 0:
                    nc.vector.tensor_copy(tm[:], lps[:])
                for i in range(HG):
                    hh = g * HG + i
                    nc.tensor.matmul(qps[i][:],
                                     bq_r[:, 128 * hh:128 * (hh + 1)],
                                     tm[0:16, :], start=False, stop=True)
                nc.tensor.matmul(vps[:], bv_r[32:48, 128 * g:128 * (g + 1)],
                                 tm[32:48, :], start=False, stop=True)
                # epilogues: RoPE q/k, transpose v into vng
                for i in range(HG):
                    rope_into(qps[i], cs, sn, qcur[:, g * HG + i, :])
                rope_into(kps, cs, sn, ktg[:, g, q0:q0 + T])
                vev = sp.tile([128, T], BF16, tag="vev", bufs=1)
                nc.vector.tensor_copy(vev[:], vps[:])
                for tt in range(4):
                    vtp = pp.tile([128, 128], BF16, tag="vt")
                    nc.tensor.transpose(
                        vtp[:], vev[:, 128 * tt:128 * (tt + 1)], ident[:])
                    nc.vector.tensor_copy(vng[:, g, 4 * t + tt, :], vtp[:])

        if t < NT - 1:
            # prefetch next hst chunk now: queued ahead of the attention
            # spill DMA, so it starts as soon as pass B's reads finish and
            # lands during chunk t's attention.
            nq0 = q0 + T
            nc.sync.dma_start(hck[:, :, :], hst_p[:, :, nq0:nq0 + T])
        if t == NT - 1:
            wctx.close()   # free wq/wk/wv/hst region for wo + ag readback
            # open the out-proj pool and start its big loads now so they
            # run during chunk 3's attention
            op = opctx.enter_context(tc.tile_pool(name=f"op{rep}", bufs=1))
            wo_r = op.tile([128, NHC, 8, 128], BF16)
            wo_p = d["wo"].rearrange("(c p) n -> p c n", p=128)
            for cc in range(0, NHC, 8):
                dst = wo_r[:, cc:cc + 8, :, :].rearrange("p a b c -> p a (b c)")
                nc.sync.dma_start(dst, wo_p[:, cc:cc + 8, :])
            if upto == "full":
                agb0 = op.tile([128, 4 * NHQ, T], BF16, tag="agb", bufs=2)
                nc.sync.dma_start(agb0[:], ag[0].rearrange("h p s -> p h s"))

        if upto == "proj":
            continue

        # ---------------- attention for chunk t ----------------
        # Software-pipelined emission (depth 3): the AV/denominator matmuls
        # for tile k flush after the scores matmuls of tiles k+1..k+3, so
        # the exp+mask latency never head-of-line-blocks the PE queue.
        with tc.tile_pool(name=f"ap{rep}_{t}", bufs=1, space="PSUM") as ap:
            spl = sp.tile([128, NHQ, T], BF16, tag="spl", bufs=1)
            kts = ktiles_for(q0)
            flat = [(h, ki, k0) for h in range(NHQ)
                    for ki, k0 in enumerate(kts)]
            avps, dnps = {}, {}
            pend = []

            def finish_head(h):
                rc = sp.tile([1, T], F32, tag="rc", bufs=2)
                nc.vector.reciprocal(rc[:], dnps[h][:])
                bc = sp.tile([128, T], F32, tag="bc", bufs=1)
                nc.gpsimd.partition_broadcast(bc[:], rc[:])
                nc.vector.tensor_mul(spl[:, h, :], avps[h][:], bc[:])

            def flush_one():
                h, k0, first, last, at = pend.pop(0)
                nc.tensor.matmul(avps[h][:], vng[:, h // HG, k0 // 128, :],
                                 at[:], start=first, stop=last)
                nc.tensor.matmul(dnps[h][:], ones_bf[:], at[:],
                                 start=first, stop=last)
                if last:
                    finish_head(h)

            for h, ki, k0 in flat:
                first, last = ki == 0, ki == len(kts) - 1
                if first:
                    avps[h] = ap.tile([128, T], F32, tag="avp", bufs=2,
                                      name=f"avp{h}")
                    dnps[h] = ap.tile([1, T], F32, tag="dnp", bufs=2,
                                      name=f"dnp{h}")
                sps = ap.tile([128, T], F32, tag="sps", bufs=4)
                nc.tensor.matmul(sps[:], ktg[:, h // HG, k0:k0 + 128],
                                 qcur[:, h, :], start=True, stop=True)
                at = sp.tile([128, T], BF16, tag="at", bufs=4)
                nc.scalar.activation(at[:], sps[:], AF.Exp)
                d0 = q0 - k0
                if d0 - 127 < 0:
                    # zero where (qq - kk + d0) < 0  (causal)
                    nc.gpsimd.affine_select(
                        out=at[:], in_=at[:], pattern=[[1, T]],
                        compare_op=mybir.AluOpType.is_ge,
                        fill=0.0, base=d0, channel_multiplier=-1)
                if d0 + T - 1 > WIN - 1:
                    # zero where (qq - kk + d0) > WIN-1 (window)
                    nc.gpsimd.affine_select(
                        out=at[:], in_=at[:], pattern=[[-1, T]],
                        compare_op=mybir.AluOpType.is_ge,
                        fill=0.0, base=WIN - 1 - d0, channel_multiplier=1)
                pend.append((h, k0, first, last, at))
                if len(pend) > 3:
                    flush_one()
            while pend:
                flush_one()
            nc.scalar.dma_start(spill[t].rearrange("h p s -> p h s"), spl[:])
            if upto == "full":
                nc.gpsimd.collective_compute(
                    "AllGather", mybir.AluOpType.bypass,
                    replica_groups=[[0, 1, 2, 3], [4, 5, 6, 7]],
                    ins=[spill[t].opt()], outs=[ag[t].opt()])

    if upto != "full":
        opctx.close()
        pactx.close()
        spctx.close()
        return

    # ---------------- output projection ----------------
    with tc.tile_pool(name=f"ops{rep}", bufs=1, space="PSUM") as opp:
        for t in range(NT):
            q0 = t * T
            if t == 0:
                agb = agb0
            else:
                agb = op.tile([128, 4 * NHQ, T], BF16, tag="agb", bufs=2)
                nc.sync.dma_start(agb[:], ag[t].rearrange("h p s -> p h s"))
            psums = [opp.tile([128, T], F32, tag=f"o{oc}", name=f"ops{oc}")
                     for oc in range(8)]
            for H in range(4 * NHQ):
                for oc in range(8):
                    nc.tensor.matmul(psums[oc][:], wo_r[:, H, oc, :],
                                     agb[:, H, :], start=(H == 0),
                                     stop=(H == 4 * NHQ - 1))
            for oc in range(8):
                ev = op.tile([128, T], F32, tag="ev", bufs=2, name=f"ev{oc}")
                nc.scalar.copy(ev[:], psums[oc][:])
                nc.sync.dma_start(out[128 * oc:128 * (oc + 1), q0:q0 + T],
                                  ev[:])
    opctx.close()
    pactx.close()
    spctx.close()


def prep_inputs(inputs):
    hs = np.asarray(inputs["hidden_states"], dtype=np.float32)
    pos = np.asarray(inputs["position_ids"]).astype(np.float64)
    Wq = np.asarray(inputs["Wq"], dtype=np.float32)
    Wk = np.asarray(inputs["Wk"], dtype=np.float32)
    Wv = np.asarray(inputs["Wv"], dtype=np.float32)
    Wo = np.asarray(inputs["Wo"], dtype=np.float32)
    aq = np.asarray(inputs["lora_A_q"], dtype=np.float32)
    bq = np.asarray(inputs["lora_B_q"], dtype=np.float32)
    av = np.asarray(inputs["lora_A_v"], dtype=np.float32)
    bv = np.asarray(inputs["lora_B_v"], dtype=np.float32)

    wq_eff = (Wq * SCALE).astype(BFDT)
    bq_eff = (bq * (LORA_SCALING * SCALE)).astype(BFDT)
    bv_eff = (bv * LORA_SCALING).astype(BFDT)
    wk_b = Wk.astype(BFDT)
    wv_b = Wv.astype(BFDT)
    wo_b = Wo.astype(BFDT)
    aq_b = aq.astype(BFDT)
    av_b = av.astype(BFDT)

    inv_freq = 1.0 / (10000.0 ** (np.arange(0, D, 2, dtype=np.float64) / D))
    tabs = []
    for b in range(2):
        freqs = np.outer(pos[b], inv_freq)          # [S, 64]
        ct = np.cos(freqs).T.astype(np.float32)
        st = np.sin(freqs).T.astype(np.float32)
        # replicated to 128 rows so each rope half reads an aligned copy
        tabs.append((np.ascontiguousarray(np.concatenate([ct, ct], axis=0)),
                     np.ascontiguousarray(np.concatenate([st, st], axis=0))))
    hsT = [np.ascontiguousarray(hs[b].T).astype(BFDT) for b in range(2)]

    in_maps = []
    for c in range(8):
        b, s = divmod(c, 4)
        cos_b, sin_b = tabs[b]
        in_maps.append({
            "hst": hsT[b],
            "wq": np.ascontiguousarray(wq_eff[:, 1024 * s:1024 * (s + 1)]),
            "wk": np.ascontiguousarray(wk_b[:, 256 * s:256 * (s + 1)]),
            "wv": np.ascontiguousarray(wv_b[:, 256 * s:256 * (s + 1)]),
            "wo": np.ascontiguousarray(wo_b[:, 1024 * s:1024 * (s + 1)]),
            "aq": aq_b, "av": av_b,
            "bq": np.ascontiguousarray(bq_eff[:, 1024 * s:1024 * (s + 1)]),
            "bv": np.ascontiguousarray(bv_eff[:, 256 * s:256 * (s + 1)]),
            "cost": cos_b, "sint": sin_b,
        })
    return in_maps


def assemble(results):
    out = np.empty((2, S, HID), dtype=np.float32)
    for c in range(8):
        b, r = divmod(c, 4)
        out[b, :, 1024 * r:1024 * (r + 1)] = results[c]["out"].T
    return out


def run_prepped(in_maps, null=False, iters=1):
    nc = build_nc(null=null, iters=iters)
    return run_bass_kernel_spmd(nc, in_maps, list(range(8)), trace=False)


def kernel(**inputs) -> np.ndarray:
    in_maps = prep_inputs(inputs)
    res = run_prepped(in_maps)
    return assemble(res.results)


# revision 4
# speedup vs baseline: 269.7932x; 1.0843x over previous
"""Mistral sliding-window GQA attention + LoRA on 8 trn2 cores — v2.

Sharding: DP2 x TP4. Core c -> batch b=c//4, head-slot s=c%4.
Each core: 8 q heads (2 kv groups of 4), full 2048-token sequence.

v2 layout: bf16 weights + activations (psum accumulates fp32), single pass
over hidden_states (resident 4MB bf16 chunk, two psum passes), attention
interleaved per 512-token chunk against an appended K/V cache, per-chunk
AllGather of bf16 attention outputs overlapped with later chunks, and the
out-projection of chunks 0-2 overlapping the final gather. Scores/q/k stay
fp32r for precision; exp/softmax denominator via ones-matmul as before.
"""
import math
from contextlib import ExitStack

import numpy as np
import ml_dtypes

import concourse.bass as bass
import concourse.mybir as mybir
import concourse.tile as tile
from concourse import bacc
from concourse.bass_utils import run_bass_kernel_spmd
from concourse.masks import make_identity

F32 = mybir.dt.float32
F32R = mybir.dt.float32r
BF16 = mybir.dt.bfloat16
AF = mybir.ActivationFunctionType
BFDT = ml_dtypes.bfloat16

HID = 4096
S = 2048
D = 128
WIN = 1024
NHQ = 8          # q heads per core
G = 2            # kv groups per core
HG = 4           # q heads per kv group
T = 512          # token chunk
NT = S // T      # 4
NHC = HID // 128  # 32 hidden chunks
NKT = S // 128    # 16 k tiles
LORA_R = 16
SCALE = 1.0 / math.sqrt(D)
LORA_SCALING = 2.0


def ktiles_for(q0):
    return [k0 for k0 in range(0, S, 128) if -(T - 128) <= q0 - k0 <= WIN]


_CACHE = {}


def build_nc(null=False, iters=1, upto="full"):
    key = ("null" if null else "full", iters, upto)
    if key in _CACHE:
        return _CACHE[key]
    nc = bacc.Bacc("TRN2", target_bir_lowering=False, debug=False,
                   num_devices=8)
    d = {}
    for name, shape, dt in [
        ("hst", [HID, S], BF16), ("wq", [HID, 1024], BF16),
        ("wk", [HID, 256], BF16), ("wv", [HID, 256], BF16),
        ("wo", [HID, 1024], BF16), ("aq", [HID, LORA_R], BF16),
        ("bq", [LORA_R, 1024], BF16), ("av", [HID, LORA_R], BF16),
        ("bv", [LORA_R, 256], BF16), ("cost", [128, S], F32),
        ("sint", [128, S], F32),
    ]:
        d[name] = nc.dram_tensor(name, shape, dt, kind="ExternalInput").ap()
    out = nc.dram_tensor("out", [1024, S], F32, kind="ExternalOutput").ap()

    if null:
        _build_null(nc, d, out)
    else:
        _build_body(nc, d, out, iters, upto)
    nc.compile()
    _CACHE[key] = nc
    return nc


def _build_null(nc, d, out):
    with tile.TileContext(nc) as tc:
        with tc.tile_pool(name="sb", bufs=2) as sb:
            t = sb.tile([64, S], F32)
            nc.sync.dma_start(t[:], d["cost"][0:64, :])
            for i in range(8):
                nc.sync.dma_start(out[128 * i:128 * i + 64, :], t[:])


def _build_body(nc, d, out, iters=1, upto="full"):
    with tile.TileContext(nc) as tc, ExitStack() as octx:
        cp = octx.enter_context(tc.tile_pool(name="const", bufs=1))
        dp = octx.enter_context(tc.tile_pool(name="dram", bufs=1, space="DRAM"))

        ident = cp.tile([128, 128], BF16)
        make_identity(nc, ident[:])
        ones_bf = cp.tile([128, 1], BF16)
        nc.gpsimd.memset(ones_bf[:], 1.0)

        spill = dp.tile([NT, NHQ, 128, T], BF16)
        ag = dp.tile([NT, 4 * NHQ, 128, T], BF16)

        for rep in range(iters):
            _one_rep(nc, tc, d, out, rep, ident, ones_bf, spill, ag, upto)


def _one_rep(nc, tc, d, out, rep, ident, ones_bf, spill, ag, upto="full"):
    # LIFO pool stack: sp (whole rep) < pa (until out-proj) < wp (until chunk 3)
    spctx = ExitStack()
    sp = spctx.enter_context(tc.tile_pool(name=f"sp{rep}", bufs=1))
    pactx = ExitStack()
    pa = pactx.enter_context(tc.tile_pool(name=f"pa{rep}", bufs=1))
    wctx = ExitStack()
    wp = wctx.enter_context(tc.tile_pool(name=f"w{rep}", bufs=1))
    opctx = ExitStack()

    # hst chunk 0 first so the first matmuls can start early; then weights.
    hck = wp.tile([128, NHC, T], BF16, tag="hck")
    hst_p = d["hst"].rearrange("(c p) s -> p c s", p=128)
    nc.sync.dma_start(hck[:, :, :], hst_p[:, :, 0:T])

    wq_r = wp.tile([128, NHC, 1024], BF16)
    wq_p = d["wq"].rearrange("(c p) n -> p c n", p=128)
    wk_r = wp.tile([128, NHC, 256], BF16)
    wk_p = d["wk"].rearrange("(c p) n -> p c n", p=128)
    wv_r = wp.tile([128, NHC, 256], BF16)
    wv_p = d["wv"].rearrange("(c p) n -> p c n", p=128)
    for cc in range(0, NHC, 8):
        nc.sync.dma_start(wq_r[:, cc:cc + 8, :], wq_p[:, cc:cc + 8, :])
        nc.sync.dma_start(wk_r[:, cc:cc + 8, :], wk_p[:, cc:cc + 8, :])
        nc.sync.dma_start(wv_r[:, cc:cc + 8, :], wv_p[:, cc:cc + 8, :])
    aq_r = wp.tile([128, NHC, LORA_R], BF16)
    nc.sync.dma_start(aq_r[:], d["aq"].rearrange("(c p) r -> p c r", p=128))
    av_r = wp.tile([128, NHC, LORA_R], BF16)
    nc.sync.dma_start(av_r[:], d["av"].rearrange("(c p) r -> p c r", p=128))
    bq_r = wp.tile([LORA_R, 1024], BF16)
    nc.sync.dma_start(bq_r[:], d["bq"][:, :])
    # rows 32:48 so the lora-v second-stage matmul shares tm's base partition
    bv_r = wp.tile([48, 256], BF16)
    nc.sync.dma_start(bv_r[32:48, :], d["bv"][:, :])

    # persistent per-rep activation state
    qcur = pa.tile([128, NHQ, T], F32R, name="qcur")
    ktg = pa.tile([128, G, S], F32R, name="ktg")
    vng = pa.tile([128, G, NKT, 128], BF16, name="vng")

    def rope_into(ps, cs, sn, dst):
        # dst = ps*cos + rotate_half(ps)*sin, psum f32 in, f32r out.
        # Stage psum -> SBUF on ACT first so the bank frees fast; the DVE
        # rope chain then runs off the critical path.
        st = sp.tile([128, T], F32, tag="rst", bufs=2)
        nc.scalar.copy(st[:], ps[:])
        c1 = sp.tile([128, T], F32, tag="rpc", bufs=1)
        nc.vector.tensor_mul(c1[0:64, :], st[0:64, :], cs[0:64, :])
        nc.vector.tensor_mul(c1[64:128, :], st[64:128, :], cs[64:128, :])
        s1 = sp.tile([128, T], F32, tag="rps", bufs=1)
        nc.vector.tensor_mul(s1[0:64, :], st[64:128, :], sn[64:128, :])
        nc.vector.tensor_mul(s1[64:128, :], st[0:64, :], sn[0:64, :])
        nc.vector.tensor_sub(dst[0:64, :], c1[0:64, :], s1[0:64, :])
        nc.vector.tensor_add(dst[64:128, :], c1[64:128, :], s1[64:128, :])

    for t in range(NT):
        q0 = t * T
        cs = sp.tile([128, T], F32, tag="cs", bufs=1)
        nc.sync.dma_start(cs[:], d["cost"][:, q0:q0 + T])
        sn = sp.tile([128, T], F32, tag="sn", bufs=1)
        nc.sync.dma_start(sn[:], d["sint"][:, q0:q0 + T])

        with tc.tile_pool(name=f"pp{rep}_{t}", bufs=1, space="PSUM") as pp:
            tm = sp.tile([48, T], BF16, tag="tm", bufs=1)
            for pas in range(2):  # pass A: heads 0-3 + kv g0 (+lora), B: 4-7 + g1
                g = pas
                qps = [pp.tile([128, T], F32, tag=f"q{i}", name=f"qps{i}")
                       for i in range(HG)]
                kps = pp.tile([128, T], F32, tag="k")
                vps = pp.tile([128, T], F32, tag="v")
                if pas == 0:
                    lps = pp.tile([48, T], F32, tag="l")
                for hc in range(NHC):
                    h = hck[:, hc, :]
                    for i in range(HG):
                        nc.tensor.matmul(
                            qps[i][:], wq_r[:, hc, 512 * g + 128 * i:
                                            512 * g + 128 * (i + 1)],
                            h, start=(hc == 0), stop=False)
                    nc.tensor.matmul(kps[:], wk_r[:, hc, 128 * g:128 * (g + 1)],
                                     h, start=(hc == 0), stop=(hc == NHC - 1))
                    nc.tensor.matmul(vps[:], wv_r[:, hc, 128 * g:128 * (g + 1)],
                                     h, start=(hc == 0), stop=False)
                    if pas == 0:
                        nc.tensor.matmul(lps[0:16, :], aq_r[:, hc, :], h,
                                         start=(hc == 0), stop=(hc == NHC - 1))
                        nc.tensor.matmul(lps[32:48, :], av_r[:, hc, :], h,
                                         start=(hc == 0), stop=(hc == NHC - 1))
                if pas == 0:
                    nc.vector.tensor_copy(tm[:], lps[:])
                for i in range(HG):
                    hh = g * HG + i
                    nc.tensor.matmul(qps[i][:],
                                     bq_r[:, 128 * hh:128 * (hh + 1)],
                                     tm[0:16, :], start=False, stop=True)
                nc.tensor.matmul(vps[:], bv_r[32:48, 128 * g:128 * (g + 1)],
                                 tm[32:48, :], start=False, stop=True)
                # epilogues: RoPE q/k, transpose v into vng
                for i in range(HG):
                    rope_into(qps[i], cs, sn, qcur[:, g * HG + i, :])
                rope_into(kps, cs, sn, ktg[:, g, q0:q0 + T])
                vev = sp.tile([128, T], BF16, tag="vev", bufs=1)
                nc.vector.tensor_copy(vev[:], vps[:])
                for tt in range(4):
                    vtp = pp.tile([128, 128], BF16, tag="vt")
                    nc.tensor.transpose(
                        vtp[:], vev[:, 128 * tt:128 * (tt + 1)], ident[:])
                    nc.vector.tensor_copy(vng[:, g, 4 * t + tt, :], vtp[:])

        if t < NT - 1:
            # prefetch next hst chunk now: queued ahead of the attention
            # spill DMA, so it starts as soon as pass B's reads finish and
            # lands during chunk t's attention.
            nq0 = q0 + T
            nc.sync.dma_start(hck[:, :, :], hst_p[:, :, nq0:nq0 + T])
        if t == NT - 1:
            wctx.close()   # free wq/wk/wv/hst region for wo + ag readback
            # open the out-proj pool and start its big loads now so they
            # run during chunk 3's attention
            op = opctx.enter_context(tc.tile_pool(name=f"op{rep}", bufs=1))
            wo_r = op.tile([128, NHC, 8, 128], BF16)
            wo_p = d["wo"].rearrange("(c p) n -> p c n", p=128)
            for cc in range(0, NHC, 8):
                dst = wo_r[:, cc:cc + 8, :, :].rearrange("p a b c -> p a (b c)")
                nc.sync.dma_start(dst, wo_p[:, cc:cc + 8, :])
            if upto == "full":
                agb0 = op.tile([128, 4 * NHQ, T], BF16, tag="agb", bufs=2)
                nc.sync.dma_start(agb0[:], ag[0].rearrange("h p s -> p h s"))

        if upto == "proj":
            continue

        # ---------------- attention for chunk t ----------------
        # Software-pipelined emission (depth 3): the AV/denominator matmuls
        # for tile k flush after the scores matmuls of tiles k+1..k+3, so
        # the exp+mask latency never head-of-line-blocks the PE queue.
        with tc.tile_pool(name=f"ap{rep}_{t}", bufs=1, space="PSUM") as ap:
            spl = sp.tile([128, NHQ, T], BF16, tag="spl", bufs=1)
            kts = ktiles_for(q0)
            flat = [(h, ki, k0) for h in range(NHQ)
                    for ki, k0 in enumerate(kts)]
            avps, dnps = {}, {}
            pend = []

            def finish_head(h):
                rc = sp.tile([1, T], F32, tag="rc", bufs=2)
                nc.vector.reciprocal(rc[:], dnps[h][:])
                bc = sp.tile([128, T], F32, tag="bc", bufs=1)
                nc.gpsimd.partition_broadcast(bc[:], rc[:])
                nc.vector.tensor_mul(spl[:, h, :], avps[h][:], bc[:])

            def flush_one():
                h, k0, first, last, at = pend.pop(0)
                nc.tensor.matmul(avps[h][:], vng[:, h // HG, k0 // 128, :],
                                 at[:], start=first, stop=last)
                nc.tensor.matmul(dnps[h][:], ones_bf[:], at[:],
                                 start=first, stop=last)
                if last:
                    finish_head(h)

            for h, ki, k0 in flat:
                first, last = ki == 0, ki == len(kts) - 1
                if first:
                    avps[h] = ap.tile([128, T], F32, tag="avp", bufs=2,
                                      name=f"avp{h}")
                    dnps[h] = ap.tile([1, T], F32, tag="dnp", bufs=2,
                                      name=f"dnp{h}")
                sps = ap.tile([128, T], F32, tag="sps", bufs=4)
                nc.tensor.matmul(sps[:], ktg[:, h // HG, k0:k0 + 128],
                                 qcur[:, h, :], start=True, stop=True)
                at = sp.tile([128, T], BF16, tag="at", bufs=5)
                nc.scalar.activation(at[:], sps[:], AF.Exp)
                d0 = q0 - k0
                if d0 - 127 < 0:
                    # zero where (qq - kk + d0) < 0  (causal)
                    nc.gpsimd.affine_select(
                        out=at[:], in_=at[:], pattern=[[1, T]],
                        compare_op=mybir.AluOpType.is_ge,
                        fill=0.0, base=d0, channel_multiplier=-1)
                if d0 + T - 1 > WIN - 1:
                    # zero where (qq - kk + d0) > WIN-1 (window)
                    nc.gpsimd.affine_select(
                        out=at[:], in_=at[:], pattern=[[-1, T]],
                        compare_op=mybir.AluOpType.is_ge,
                        fill=0.0, base=WIN - 1 - d0, channel_multiplier=1)
                pend.append((h, k0, first, last, at))
                if len(pend) > 4:
                    flush_one()
            while pend:
                flush_one()
            nc.scalar.dma_start(spill[t].rearrange("h p s -> p h s"), spl[:])
            if upto == "full":
                nc.gpsimd.collective_compute(
                    "AllGather", mybir.AluOpType.bypass,
                    replica_groups=[[0, 1, 2, 3], [4, 5, 6, 7]],
                    ins=[spill[t].opt()], outs=[ag[t].opt()])

    if upto != "full":
        opctx.close()
        pactx.close()
        spctx.close()
        return

    # ---------------- output projection ----------------
    with tc.tile_pool(name=f"ops{rep}", bufs=1, space="PSUM") as opp:
        for t in range(NT):
            q0 = t * T
            if t == 0:
                agb = agb0
            else:
                agb = op.tile([128, 4 * NHQ, T], BF16, tag="agb", bufs=2)
                nc.sync.dma_start(agb[:], ag[t].rearrange("h p s -> p h s"))
            psums = [opp.tile([128, T], F32, tag=f"o{oc}", name=f"ops{oc}")
                     for oc in range(8)]
            for H in range(4 * NHQ):
                for oc in range(8):
                    nc.tensor.matmul(psums[oc][:], wo_r[:, H, oc, :],
                                     agb[:, H, :], start=(H == 0),
                                     stop=(H == 4 * NHQ - 1))
            for oc in range(8):
                ev = op.tile([128, T], F32, tag="ev", bufs=2, name=f"ev{oc}")
                nc.scalar.copy(ev[:], psums[oc][:])
                nc.sync.dma_start(out[128 * oc:128 * (oc + 1), q0:q0 + T],
                                  ev[:])
    opctx.close()
    pactx.close()
    spctx.close()


def prep_inputs(inputs):
    hs = np.asarray(inputs["hidden_states"], dtype=np.float32)
    pos = np.asarray(inputs["position_ids"]).astype(np.float64)
    Wq = np.asarray(inputs["Wq"], dtype=np.float32)
    Wk = np.asarray(inputs["Wk"], dtype=np.float32)
    Wv = np.asarray(inputs["Wv"], dtype=np.float32)
    Wo = np.asarray(inputs["Wo"], dtype=np.float32)
    aq = np.asarray(inputs["lora_A_q"], dtype=np.float32)
    bq = np.asarray(inputs["lora_B_q"], dtype=np.float32)
    av = np.asarray(inputs["lora_A_v"], dtype=np.float32)
    bv = np.asarray(inputs["lora_B_v"], dtype=np.float32)

    wq_eff = (Wq * SCALE).astype(BFDT)
    bq_eff = (bq * (LORA_SCALING * SCALE)).astype(BFDT)
    bv_eff = (bv * LORA_SCALING).astype(BFDT)
    wk_b = Wk.astype(BFDT)
    wv_b = Wv.astype(BFDT)
    wo_b = Wo.astype(BFDT)
    aq_b = aq.astype(BFDT)
    av_b = av.astype(BFDT)

    inv_freq = 1.0 / (10000.0 ** (np.arange(0, D, 2, dtype=np.float64) / D))
    tabs = []
    for b in range(2):
        freqs = np.outer(pos[b], inv_freq)          # [S, 64]
        ct = np.cos(freqs).T.astype(np.float32)
        st = np.sin(freqs).T.astype(np.float32)
        # replicated to 128 rows so each rope half reads an aligned copy
        tabs.append((np.ascontiguousarray(np.concatenate([ct, ct], axis=0)),
                     np.ascontiguousarray(np.concatenate([st, st], axis=0))))
    hsT = [np.ascontiguousarray(hs[b].T).astype(BFDT) for b in range(2)]

    in_maps = []
    for c in range(8):
        b, s = divmod(c, 4)
        cos_b, sin_b = tabs[b]
        in_maps.append({
            "hst": hsT[b],
            "wq": np.ascontiguousarray(wq_eff[:, 1024 * s:1024 * (s + 1)]),
            "wk": np.ascontiguousarray(wk_b[:, 256 * s:256 * (s + 1)]),
            "wv": np.ascontiguousarray(wv_b[:, 256 * s:256 * (s + 1)]),
            "wo": np.ascontiguousarray(wo_b[:, 1024 * s:1024 * (s + 1)]),
            "aq": aq_b, "av": av_b,
            "bq": np.ascontiguousarray(bq_eff[:, 1024 * s:1024 * (s + 1)]),
            "bv": np.ascontiguousarray(bv_eff[:, 256 * s:256 * (s + 1)]),
            "cost": cos_b, "sint": sin_b,
        })
    return in_maps


def assemble(results):
    out = np.empty((2, S, HID), dtype=np.float32)
    for c in range(8):
        b, r = divmod(c, 4)
        out[b, :, 1024 * r:1024 * (r + 1)] = results[c]["out"].T
    return out


def run_prepped(in_maps, null=False, iters=1):
    nc = build_nc(null=null, iters=iters)
    return run_bass_kernel_spmd(nc, in_maps, list(range(8)), trace=False)


def kernel(**inputs) -> np.ndarray:
    in_maps = prep_inputs(inputs)
    res = run_prepped(in_maps)
    return assemble(res.results)
